# revision 13
# baseline (speedup 1.0000x reference)
"""TRN2 Bass kernel for nn_ChartOperator (sparse_attention).

Math (B=4, N=4096, PD=1024, D=16, S=64, ALL=1024):
  P = x @ W_r + b_r
  L = P[..., :ALL].reshape(n, D, S); R = P[..., ALL:].reshape(n, D, S)
  w = softmax_a(L)
  Q[n, d, s] = sum_{a<d} w[n,a,s] * R[n+a+1, d-1-a, s]
  (last D rows of each batch: Q[t+d>=16] zeroed)
  out = Q.reshape(n, ALL) @ W_w + b_w

Sharding: 8 cores, data-parallel over flattened (B*N) rows, 2048 rows/core
plus a 128-row forward halo (zero-padded at batch boundaries; the affected
outputs are exactly the masked ones).

Per-core pipeline (bf16 matmuls, fp32 PSUM):
  1. Reader computes P TRANSPOSED: psum[(d2,s64), n512] = W_r_slab.T @ xT
     (bias via K=1 matmul with b_r slab as stationary, ones moving).
  2. ACT exp/copy writes the banded-MAC layout directly:
     e chain  et_all[(g2,s64), a16, n1024]   (g: row-blocks 0-7 / 8-15)
     r chain  rt_all[(g2,s64), c16, n1152]   (blocks 0-8 / 8-16 incl halo)
  3. DVE: Z = reduce over a; reciprocal; normalize et_all in place.
  4. DVE banded products p[s, c, n] = w[s,a,n]*r[s,c,n+a+1] (single op per a)
  5. PE identity-matmuls accumulate products into PSUM Q[(g,s), d, n]
     with shrinking d-windows (d = a+c+1)
  6. ACT strided copies Q -> qt[(dsub2,s64), k8, n128] bf16 (writer lhsT)
  7. writer matmuls out[n128, 1024] = qt.T @ W_w + b_w -> DMA out
"""
import numpy as np
import ml_dtypes
from contextlib import ExitStack

import concourse.bass as bass
import concourse.tile as tile
from concourse import bacc, mybir
from concourse import bass_utils

BF16 = mybir.dt.bfloat16
F32 = mybir.dt.float32
bfnp = ml_dtypes.bfloat16

B, N, PD = 4, 4096, 1024
D, S = 16, 64
ALL = D * S
ROWS_PER_CORE = 2048
NROWS = 2176                   # + 128-row halo
NCP = 8

_cache = {}


def _build(debug=False):
    nc = bacc.Bacc("TRN2", target_bir_lowering=False, debug=False, num_devices=8)

    xT_d = nc.dram_tensor("xT", [8, 128, NROWS], BF16, kind="ExternalInput").ap()
    wr_d = nc.dram_tensor("wr", [8, 128, 2048], BF16, kind="ExternalInput").ap()
    ww_d = nc.dram_tensor("ww", [8, 128, 1024], BF16, kind="ExternalInput").ap()
    br_d = nc.dram_tensor("br", [128, 16], F32, kind="ExternalInput").ap()
    ident_d = nc.dram_tensor("ident", [128, 128], BF16, kind="ExternalInput").ap()
    qmask_d = nc.dram_tensor("qmask", [128, 8, 128], BF16, kind="ExternalInput").ap()
    out_d = nc.dram_tensor("out", [16, 128, 1024], F32, kind="ExternalOutput").ap()

    with tile.TileContext(nc) as tc, ExitStack() as ctx:
        cpool = ctx.enter_context(tc.tile_pool(name="cpool", bufs=1))
        ps512 = ctx.enter_context(tc.tile_pool(name="ps512", bufs=2, space="PSUM"))
        wps = ctx.enter_context(tc.tile_pool(name="wps", bufs=2, space="PSUM"))
        ztp = ctx.enter_context(tc.tile_pool(name="ztp", bufs=1))
        rzp = ctx.enter_context(tc.tile_pool(name="rzp", bufs=1))
        prodp = ctx.enter_context(tc.tile_pool(name="prodp", bufs=4))
        macp = ctx.enter_context(tc.tile_pool(name="macp", bufs=1, space="PSUM"))
        qtp = ctx.enter_context(tc.tile_pool(name="qtp", bufs=3))
        osbp = ctx.enter_context(tc.tile_pool(name="osbp", bufs=2))

        # --- persistent constants / big buffers
        xk = [cpool.tile([128, NROWS], BF16, name=f"xk{i}", tag=f"xk{i}")
              for i in range(8)]
        for ks in range(8):
            nc.gpsimd.dma_start(xk[ks][:], xT_d[ks])
        wr_sb = cpool.tile([128, 8, 2048], BF16)
        nc.gpsimd.dma_start(wr_sb[:], wr_d[:].rearrange("k p c -> p k c"))
        ww_sb = cpool.tile([128, 8, 1024], BF16)
        nc.gpsimd.dma_start(ww_sb[:], ww_d[:].rearrange("k p c -> p k c"))
        br_sb = cpool.tile([128, 16], F32)
        nc.gpsimd.dma_start(br_sb[:], br_d[:])
        ident = cpool.tile([128, 128], BF16)
        nc.gpsimd.dma_start(ident[:], ident_d[:])
        qmask = cpool.tile([128, 8, 128], BF16)
        nc.gpsimd.dma_start(qmask[:], qmask_d[:])
        et_all = cpool.tile([128, 16, 1024], BF16)   # [(g2,s64), a, n-chain]
        rt_all = cpool.tile([128, 16, 1152], BF16)   # [(g2,s64), c, n-chain]

        # ---------------- Loop 1: transposed reader + layout writes
        # jj: 4 supertiles of 512 rows + 1 halo tile of 128 rows
        for jj in (0, 2, 1, 3, 4):
            nwin = 128 if jj == 4 else 512
            n0 = jj * 512
            g = 0 if jj < 2 else 1
            for u in range(16):               # col slabs: 0-7 = L, 8-15 = R
                is_l = u < 8
                if is_l and jj == 4:
                    continue                  # halo rows: R only
                ps = ps512.tile([128, 512], F32, tag="ps512", name="ps")
                for ks in range(8):
                    nc.tensor.matmul(ps[:, :nwin], wr_sb[:, ks, 128 * u:128 * (u + 1)],
                                     xk[ks][:, n0:n0 + nwin],
                                     start=(ks == 0), stop=(ks == 7))
                for dsub in range(2):
                    src = ps[64 * dsub:64 * dsub + 64, :nwin]
                    bias = br_sb[64 * dsub:64 * dsub + 64, u:u + 1]
                    AF = mybir.ActivationFunctionType
                    if is_l:
                        a = 2 * u + dsub
                        dst = et_all[64 * g:64 * g + 64, a,
                                     n0 - 1024 * g:n0 - 1024 * g + nwin]
                        nc.scalar.activation(dst, src, AF.Exp, bias=bias)
                    else:
                        c = 2 * (u - 8) + dsub
                        # g0 chain: blocks 0..8 ; g1 chain: blocks 8..16
                        if jj < 2:
                            nc.scalar.activation(rt_all[0:64, c, n0:n0 + 512], src,
                                                 AF.Identity, bias=bias)
                        elif jj == 2:   # blocks 8-11: both chains
                            nc.scalar.activation(rt_all[0:64, c, 1024:1152],
                                                 ps[64 * dsub:64 * dsub + 64, 0:128],
                                                 AF.Identity, bias=bias)
                            nc.scalar.activation(rt_all[64:128, c, 0:512], src,
                                                 AF.Identity, bias=bias)
                        elif jj == 3:
                            nc.scalar.activation(rt_all[64:128, c, 512:1024], src,
                                                 AF.Identity, bias=bias)
                        else:           # halo block 16
                            nc.scalar.activation(rt_all[64:128, c, 1024:1152], src,
                                                 AF.Identity, bias=bias)

            if jj in (2, 3):
                # normalize chain-window: w0 = supertiles {0,2}, w1 = {1,3}
                # (loop order 0,2,1,3 makes w0 ready after the 2nd supertile)
                lo = (jj - 2) * 512
                zt = ztp.tile([128, 512], F32, tag="ztp", name="zt")
                e_na = et_all[:, :, lo:lo + 512].rearrange("p a n -> p n a")
                nc.vector.tensor_reduce(zt[:], e_na, axis=mybir.AxisListType.X,
                                        op=mybir.AluOpType.add)
                rz = rzp.tile([128, 512], F32, tag="rzp", name="rz")
                nc.vector.reciprocal(rz[:], zt[:])
                rz3 = rz[:].rearrange("p (o n) -> p o n", o=1).to_broadcast((128, 16, 512))
                ew = et_all[:, :, lo:lo + 512]
                nc.vector.tensor_mul(ew, ew, rz3)

        # ---------------- Loop 2+3: banded MAC + writer per chunklet-pair
        for cp in range(NCP):
            mp = macp.tile([128, 16, 128], F32, tag="macp", name="mp")
            nc.vector.memset(mp[:, 0, :], 0.0)
            n0 = 128 * cp
            for a in range(15):
                cnt = 15 - a
                p = prodp.tile([128, 15, 128], BF16, tag="prodp", name="p")
                eb = et_all[:, a:a + 1, n0:n0 + 128].to_broadcast((128, cnt, 128))
                # odd (a+1)-shifts run at DVE 1x (4B misalignment); split
                # those across DVE and GpSimd to balance the two engines
                if a % 2 == 0 and cnt >= 2:
                    cs = max(1, (3 * cnt) // 5)
                    nc.vector.tensor_mul(p[:, 0:cs, :], eb[:, 0:cs, :],
                                         rt_all[:, 0:cs, n0 + a + 1:n0 + a + 129])
                    nc.gpsimd.tensor_mul(p[:, cs:cnt, :], eb[:, 0:cnt - cs, :],
                                         rt_all[:, cs:cnt, n0 + a + 1:n0 + a + 129])
                else:
                    nc.vector.tensor_mul(p[:, 0:cnt, :], eb,
                                         rt_all[:, 0:cnt, n0 + a + 1:n0 + a + 129])
                for b in range(4):
                    d_lo = max(a + 1, 4 * b)
                    d_hi = 4 * b + 4
                    if d_lo >= d_hi:
                        continue
                    last_a = min(14, 4 * b + 2)
                    nc.tensor.matmul(mp[:, d_lo:d_hi, :], ident[:],
                                     p[:, d_lo - a - 1:d_hi - a - 1, :],
                                     start=(a == 0), stop=(a == last_a))

            for g in range(2):
                cb = 8 * g + cp
                qt = qtp.tile([128, 8, 128], BF16, tag="qtp", name="qt")
                for dsub in range(2):
                    csrc = mp[64 * g:64 * g + 64, dsub::2, :]
                    cdst = qt[64 * dsub:64 * dsub + 64, :, :]
                    if g == 0:
                        nc.scalar.copy(cdst, csrc)
                    else:
                        nc.vector.tensor_copy(cdst, csrc)
                if cb == 15:
                    nc.vector.tensor_mul(qt[:], qt[:], qmask[:])
                osb = osbp.tile([128, 1024], F32, tag="osbp", name="osb")
                for h in range(2):
                    wp = wps.tile([128, 512], F32, tag="wps", name="wp")
                    for k in range(8):
                        nc.tensor.matmul(wp[:], qt[:, k, :],
                                         ww_sb[:, k, h * 512:(h + 1) * 512],
                                         start=(k == 0), stop=(k == 7))
                    nc.vector.tensor_copy(osb[:, h * 512:(h + 1) * 512], wp[:])
                nc.gpsimd.dma_start(out_d[cb], osb[:])

    nc.compile()
    return nc


def _host_prep(x, W_r, b_r, W_w, b_w):
    """Build the 8 per-core input maps."""
    xf = np.asarray(x, np.float32).reshape(B * N, PD)
    wr = np.asarray(W_r, np.float32).astype(bfnp)
    ww = np.asarray(W_w, np.float32).astype(bfnp)
    br = np.ascontiguousarray(
        np.asarray(b_r, np.float32).reshape(16, 128).T)
    wr_t = np.ascontiguousarray(wr.reshape(8, 128, 2048))
    ww_t = np.ascontiguousarray(ww.reshape(8, 128, 1024))
    ident = np.eye(128, dtype=np.float32).astype(bfnp)

    in_maps = []
    for c in range(8):
        lo = c * ROWS_PER_CORE
        chunk = np.zeros((NROWS, PD), np.float32)
        chunk[:ROWS_PER_CORE] = xf[lo:lo + ROWS_PER_CORE]
        if c % 2 == 0:
            chunk[ROWS_PER_CORE:] = xf[lo + ROWS_PER_CORE: lo + NROWS]
        # xT[ks, k, n] = chunk[n, 128*ks + k]
        xt = np.ascontiguousarray(
            chunk.astype(bfnp).reshape(NROWS, 8, 128).transpose(1, 2, 0))
        qmask = np.ones((128, 8, 128), np.float32)
        if c % 2 == 1:
            dsub = (np.arange(128)[:, None, None] // 64)
            k = np.arange(8)[None, :, None]
            n = np.arange(128)[None, None, :]
            bad = (n >= 112) & ((n - 112 + 2 * k + dsub) >= 16)
            qmask[np.broadcast_to(bad, (128, 8, 128))] = 0.0
        in_maps.append({
            "xT": xt,
            "wr": wr_t, "ww": ww_t, "br": br,
            "ident": ident, "qmask": qmask.astype(bfnp),
        })
    return in_maps


def kernel(x, W_r, b_r, W_w, b_w):
    if "nc" not in _cache:
        _cache["nc"] = _build()
    nc = _cache["nc"]
    in_maps = _host_prep(x, W_r, b_r, W_w, b_w)
    res = bass_utils.run_bass_kernel_spmd(nc, in_maps, core_ids=list(range(8)))
    out = np.concatenate([r["out"].reshape(ROWS_PER_CORE, ALL)
                          for r in res.results], axis=0)
    out = out.reshape(B, N, ALL).astype(np.float32)
    out += np.asarray(b_w, np.float32).reshape(1, 1, ALL)
    return np.ascontiguousarray(out)
